# revision 10
# baseline (speedup 1.0000x reference)
# kernel.py — self-contained Trainium2 Bass kernel for nn_AttFlat
# Contract: kernel(**inputs: np.ndarray) -> np.ndarray with FULL inputs/output.
# Internally shards batch across 8 NeuronCores (pure data parallel).
import math
import os
import sys

import numpy as np

for _p in ("/opt/trn_rl_repo", "/root/.axon_site/_ro/trn_rl_repo"):
    if os.path.isdir(_p) and _p not in sys.path:
        sys.path.insert(0, _p)

# ---------------- problem constants (hardcoded from spec) ----------------
B, L, H, M, GL, O = 256, 196, 1024, 512, 1, 2048
NCORES = 8
BPC = B // NCORES          # 32 samples per core
ROWS = BPC * L             # 6272 rows per core
NB = 100                   # basis functions
SEQ_SIDE = 14
RIDGE = 0.5
BASIS_VAR = 0.001
TWO_PI = 2.0 * math.pi
EPS_COV = 1e-6
DIAG_ADD = EPS_COV + BASIS_VAR  # added to Sigma diagonal before inversion

CH = 4                     # samples per chunk
NCHUNK = BPC // CH         # 8 chunks per core
CROWS = CH * L             # 784 rows per chunk
NKT = (CROWS + 127) // 128  # 7 k-tiles per chunk (6x128 + 16)
TAIL = CROWS - 6 * 128      # 16

_F32 = np.float32


def _host_constants():
    """pos, mus, G computed exactly like the reference (jax f32 on CPU if
    available so the f32 matrix inverse matches; falls back to numpy)."""
    s = 1.0 / SEQ_SIDE
    lin = np.linspace(s, 1.0 - s, SEQ_SIDE)
    px, py = np.meshgrid(lin, lin, indexing="ij")
    pos = np.stack([px.ravel(), py.ravel()], axis=-1).astype(_F32)  # [L,2]

    lin2 = np.linspace(0.0, 1.0, int(math.sqrt(NB)))
    mx, my = np.meshgrid(lin2, lin2, indexing="ij")
    mus = np.stack([mx.ravel(), my.ravel()], axis=-1).astype(_F32)  # [NB,2]

    G = None
    try:
        import jax

        cpu = jax.devices("cpu")[0]
        with jax.default_device(cpu):
            import jax.numpy as jnp

            posj = jnp.asarray(pos)
            musj = jnp.asarray(mus)
            d2 = jnp.sum((posj[None, :, :] - musj[:, None, :]) ** 2, axis=-1)
            F = jnp.exp(-0.5 * d2 / BASIS_VAR) / (TWO_PI * BASIS_VAR)
            A = F @ F.T + RIDGE * jnp.eye(NB, dtype=F.dtype)
            Gj = F.T @ jnp.linalg.inv(A)
            G = np.asarray(Gj, dtype=_F32)
    except Exception:
        pass
    if G is None:
        d2 = np.sum(
            (pos[None, :, :].astype(_F32) - mus[:, None, :].astype(_F32)) ** 2,
            axis=-1,
        ).astype(_F32)
        F = (np.exp(-0.5 * d2 / BASIS_VAR) / (TWO_PI * BASIS_VAR)).astype(_F32)
        A = (F @ F.T + RIDGE * np.eye(NB, dtype=_F32)).astype(_F32)
        G = (F.T @ np.linalg.inv(A)).astype(_F32)
    return pos, mus, G  # [L,2], [NB,2], [L,NB]


_MODULE_CACHE = {}


def _build_module():
    """Build (and cache) the Bass module for one NeuronCore."""
    if "nc" in _MODULE_CACHE:
        return _MODULE_CACHE["nc"]

    import concourse.bass as bass
    import concourse.mybir as mybir
    import concourse.tile as tile
    from concourse import bacc
    from contextlib import ExitStack

    f32 = mybir.dt.float32
    f32r = mybir.dt.float32r
    AX = mybir.AxisListType
    AF = mybir.ActivationFunctionType

    pos, mus, G = _host_constants()
    Gt = np.ascontiguousarray(G.T)                      # [NB, L]
    posx_c = np.broadcast_to(pos[:, 0], (CH, L)).copy() # [CH, L]
    posy_c = np.broadcast_to(pos[:, 1], (CH, L)).copy()
    musx_c = np.broadcast_to(mus[:, 0], (CH, NB)).copy()
    musy_c = np.broadcast_to(mus[:, 1], (CH, NB)).copy()
    bmask = np.zeros((CH, CROWS), dtype=_F32)           # block mask
    for s in range(CH):
        bmask[s, s * L:(s + 1) * L] = 1.0
    ident_np = np.eye(128, dtype=_F32)
    ones_np = np.ones((1, BPC), dtype=_F32)

    nc = bacc.Bacc("TRN2", target_bir_lowering=False)

    x_d = nc.dram_tensor("x", [ROWS, H], f32, kind="ExternalInput")
    negm_d = nc.dram_tensor("negmask", [BPC, L], f32, kind="ExternalInput")
    W1_d = nc.dram_tensor("W1", [H, M], f32, kind="ExternalInput")
    b1r_d = nc.dram_tensor("b1r", [128, 4], f32, kind="ExternalInput")
    W2r_d = nc.dram_tensor("W2r", [128, 4], f32, kind="ExternalInput")
    Wm_d = nc.dram_tensor("Wm", [H, O], f32, kind="ExternalInput")
    bm_d = nc.dram_tensor("bm", [1, O], f32, kind="ExternalInput")
    out_d = nc.dram_tensor("out", [BPC, O], f32, kind="ExternalOutput")

    ident_d = nc.inline_tensor(ident_np, "ident_c")
    gt_d = nc.inline_tensor(Gt, "gt_c")
    posx_d = nc.inline_tensor(posx_c, "posx_c")
    posy_d = nc.inline_tensor(posy_c, "posy_c")
    musx_d = nc.inline_tensor(musx_c, "musx_c")
    musy_d = nc.inline_tensor(musy_c, "musy_c")
    bmask_d = nc.inline_tensor(bmask, "bmask_c")
    ones_d = nc.inline_tensor(ones_np, "ones_c")

    def r32(ap):
        return ap.bitcast(f32r)

    with tile.TileContext(nc) as tc, ExitStack() as ctx:
        consts = ctx.enter_context(tc.tile_pool(name="consts", bufs=1))
        xnat = ctx.enter_context(tc.tile_pool(name="xnat", bufs=8))
        xtp = ctx.enter_context(tc.tile_pool(name="xtp", bufs=1))
        htp = ctx.enter_context(tc.tile_pool(name="htp", bufs=2))
        sm = ctx.enter_context(tc.tile_pool(name="sm", bufs=1))
        accp = ctx.enter_context(tc.tile_pool(name="accp", bufs=1))
        psum_t = ctx.enter_context(tc.tile_pool(name="psum_t", bufs=2, space="PSUM"))
        psum_h = ctx.enter_context(tc.tile_pool(name="psum_h", bufs=2, space="PSUM"))
        psum_l = ctx.enter_context(tc.tile_pool(name="psum_l", bufs=1, space="PSUM"))
        psum_s = ctx.enter_context(tc.tile_pool(name="psum_s", bufs=1, space="PSUM"))
        psum_c = ctx.enter_context(tc.tile_pool(name="psum_c", bufs=2, space="PSUM"))

        # ---- load constants / weights ----
        W1_sb = consts.tile([128, 8, M], f32r)
        nc.sync.dma_start(out=W1_sb, in_=W1_d[:, :].rearrange("(k p) m -> p k m", p=128).bitcast(f32r))
        Wm_sb = consts.tile([128, 8, O], f32r)
        nc.sync.dma_start(out=Wm_sb, in_=Wm_d[:, :].rearrange("(k p) m -> p k m", p=128).bitcast(f32r))
        W2_sb = consts.tile([128, 4], f32r)
        nc.gpsimd.dma_start(out=W2_sb, in_=W2r_d[:, :].bitcast(f32r))
        b1_sb = consts.tile([128, 4], f32)
        nc.gpsimd.dma_start(out=b1_sb, in_=b1r_d[:, :])
        bm_sb = consts.tile([1, O], f32r)
        nc.gpsimd.dma_start(out=bm_sb, in_=bm_d[:, :].bitcast(f32r))
        Gt_sb = consts.tile([NB, L], f32r)
        nc.gpsimd.dma_start(out=Gt_sb, in_=gt_d[:, :].bitcast(f32r))
        ident = consts.tile([128, 128], f32)
        nc.gpsimd.dma_start(out=ident, in_=ident_d[:, :])
        posx_sb = consts.tile([CH, L], f32)
        nc.gpsimd.dma_start(out=posx_sb, in_=posx_d[:, :])
        posy_sb = consts.tile([CH, L], f32)
        nc.gpsimd.dma_start(out=posy_sb, in_=posy_d[:, :])
        musx_sb = consts.tile([CH, NB], f32)
        nc.gpsimd.dma_start(out=musx_sb, in_=musx_d[:, :])
        musy_sb = consts.tile([CH, NB], f32)
        nc.gpsimd.dma_start(out=musy_sb, in_=musy_d[:, :])
        bmask_sb = consts.tile([CH, CROWS], f32)
        nc.gpsimd.dma_start(out=bmask_sb, in_=bmask_d[:, :])
        ones_sb = consts.tile([1, BPC], f32r)
        nc.gpsimd.dma_start(out=ones_sb, in_=ones_d[:, :].bitcast(f32r))
        negm_sb = consts.tile([CH, NCHUNK, L], f32)
        nc.gpsimd.dma_start(
            out=negm_sb, in_=negm_d[:, :].rearrange("(c s) l -> s c l", s=CH)
        )
        zbias = consts.tile([CH, 1], f32)
        nc.vector.memset(zbias, 0.0)

        ctxT_sb = accp.tile([128, 8, BPC], f32r)  # ctx^T accumulated over chunks

        for c in range(NCHUNK):
            # ---- load x chunk (natural layout, row-major tiles) ----
            xts = []
            for kt in range(NKT):
                rows = 128 if kt < NKT - 1 else TAIL
                x_t = xnat.tile([128, H], f32r, tag="xnat", name=f"x_t")
                nc.sync.dma_start(
                    out=x_t[:rows, :],
                    in_=x_d[c * CROWS + kt * 128: c * CROWS + kt * 128 + rows, :].bitcast(f32r),
                )
                xts.append((x_t, rows))

            # ---- transpose x -> xT [128h x 8 x 784rows] ----
            xT = xtp.tile([128, 8, CROWS], f32r, tag="xT", name="xT")
            for hb in range(8):
                # group A: k-tiles 0..3 -> psum [128, 512]
                psA = psum_t.tile([128, 512], f32, tag="tp", name="psA")
                for kt in range(4):
                    nc.tensor.transpose(
                        out=psA[:, kt * 128:(kt + 1) * 128],
                        in_=xts[kt][0][:128, hb * 128:(hb + 1) * 128].bitcast(f32),
                        identity=ident[:128, :128],
                    )
                nc.scalar.copy(out=xT[:, hb, 0:512], in_=psA[:, :])
                # group B: k-tiles 4,5 (128) + 6 (16) -> psum [128, 272]
                psB = psum_t.tile([128, 512], f32, tag="tp", name="psB")
                off = 0
                for kt in range(4, NKT):
                    rows = xts[kt][1]
                    nc.tensor.transpose(
                        out=psB[:, off:off + rows],
                        in_=xts[kt][0][:rows, hb * 128:(hb + 1) * 128].bitcast(f32),
                        identity=ident[:rows, :rows],
                    )
                    off += rows
                nc.scalar.copy(out=xT[:, hb, 512:CROWS], in_=psB[:, :off])

            # ---- mm1 (relu(x@W1+b1)) and mm2 (@W2) per 392-row half ----
            logits_sb = sm.tile([1, CROWS], f32, tag="logits", name="logits_sb", bufs=2)
            for half in range(2):
                r0 = half * 392
                hT = htp.tile([128, 4, 392], f32r, tag="hT", name="hT")
                for mt in range(4):
                    ph = psum_h.tile([128, 392], f32, tag="hps", name="ph")
                    for k in range(8):
                        nc.tensor.matmul(
                            ph[:, :],
                            W1_sb[:, k, mt * 128:(mt + 1) * 128],
                            xT[:, k, r0:r0 + 392],
                            start=(k == 0),
                            stop=(k == 7),
                        )
                    nc.scalar.activation(
                        out=hT[:, mt, :],
                        in_=ph[:, :],
                        func=AF.Relu,
                        bias=b1_sb[:, mt:mt + 1],
                        scale=1.0,
                    )
                pl = psum_l.tile([1, 392], f32, tag="lps", name="pl")
                for mt in range(4):
                    nc.tensor.matmul(
                        pl[:, :],
                        W2_sb[:, mt:mt + 1],
                        hT[:, mt, :],
                        start=(mt == 0),
                        stop=(mt == 3),
                    )
                nc.scalar.copy(out=logits_sb[:, r0:r0 + 392], in_=pl[:, :])

            # ---- rearrange logits to sample-major [4, 196] ----
            lsm = sm.tile([CH, L], f32, tag="lsm", name="lsm")
            for s in range(CH):
                nc.gpsimd.dma_start(
                    out=lsm[s:s + 1, :], in_=logits_sb[0:1, s * L:(s + 1) * L]
                )
            # mask
            nc.vector.tensor_add(out=lsm, in0=lsm, in1=negm_sb[:, c, :])

            # ---- softmax over L ----
            mx = sm.tile([CH, 1], f32, tag="mx", name="mx")
            nc.vector.reduce_max(out=mx, in_=lsm, axis=AX.X)
            negmx = sm.tile([CH, 1], f32, tag="negmx", name="negmx")
            nc.vector.tensor_scalar_mul(negmx, mx, -1.0)
            e_sb = sm.tile([CH, L], f32, tag="e", name="e_sb")
            ssum = sm.tile([CH, 1], f32, tag="ssum", name="ssum")
            nc.scalar.activation(
                out=e_sb, in_=lsm, func=AF.Exp, bias=negmx[:, 0:1], scale=1.0,
                accum_out=ssum[:, 0:1],
            )
            rsum = sm.tile([CH, 1], f32, tag="rsum", name="rsum")
            nc.vector.reciprocal(out=rsum, in_=ssum)
            p_sb = sm.tile([CH, L], f32, tag="p", name="p_sb")
            nc.vector.tensor_scalar_mul(p_sb, e_sb, rsum[:, 0:1])

            # ---- fit gaussian: mu, Sigma ----
            tpx = sm.tile([CH, L], f32, tag="tpx", name="tpx")
            nc.vector.tensor_mul(tpx, p_sb, posx_sb)
            mux = sm.tile([CH, 1], f32, tag="mux", name="mux")
            nc.vector.reduce_sum(out=mux, in_=tpx, axis=AX.X)
            tpy = sm.tile([CH, L], f32, tag="tpy", name="tpy")
            nc.vector.tensor_mul(tpy, p_sb, posy_sb)
            muy = sm.tile([CH, 1], f32, tag="muy", name="muy")
            nc.vector.reduce_sum(out=muy, in_=tpy, axis=AX.X)

            dx = sm.tile([CH, L], f32, tag="dx", name="dx")
            nc.vector.tensor_scalar_sub(dx, posx_sb, mux[:, 0:1])
            dy = sm.tile([CH, L], f32, tag="dy", name="dy")
            nc.vector.tensor_scalar_sub(dy, posy_sb, muy[:, 0:1])
            pdx = sm.tile([CH, L], f32, tag="pdx", name="pdx")
            nc.vector.tensor_mul(pdx, p_sb, dx)
            pdy = sm.tile([CH, L], f32, tag="pdy", name="pdy")
            nc.vector.tensor_mul(pdy, p_sb, dy)

            tq = sm.tile([CH, L], f32, tag="tq", name="tq")
            sxx = sm.tile([CH, 1], f32, tag="sxx", name="sxx")
            nc.vector.tensor_mul(tq, pdx, dx)
            nc.vector.reduce_sum(out=sxx, in_=tq, axis=AX.X)
            sxy = sm.tile([CH, 1], f32, tag="sxy", name="sxy")
            nc.vector.tensor_mul(tq, pdx, dy)
            nc.vector.reduce_sum(out=sxy, in_=tq, axis=AX.X)
            syy = sm.tile([CH, 1], f32, tag="syy", name="syy")
            nc.vector.tensor_mul(tq, pdy, dy)
            nc.vector.reduce_sum(out=syy, in_=tq, axis=AX.X)

            a_sb = sm.tile([CH, 1], f32, tag="a_sb", name="a_sb")
            nc.vector.tensor_scalar_add(a_sb, sxx, DIAG_ADD)
            d_sb = sm.tile([CH, 1], f32, tag="d_sb", name="d_sb")
            nc.vector.tensor_scalar_add(d_sb, syy, DIAG_ADD)

            det = sm.tile([CH, 1], f32, tag="det", name="det")
            nc.vector.tensor_mul(det, a_sb, d_sb)
            b2t = sm.tile([CH, 1], f32, tag="b2t", name="b2t")
            nc.vector.tensor_mul(b2t, sxy, sxy)
            nc.vector.tensor_sub(det, det, b2t)
            rdet = sm.tile([CH, 1], f32, tag="rdet", name="rdet")
            nc.vector.reciprocal(out=rdet, in_=det)

            sxxi = sm.tile([CH, 1], f32, tag="sxxi", name="sxxi")
            nc.vector.tensor_mul(sxxi, d_sb, rdet)   # quad coeff for dmx^2
            syyi = sm.tile([CH, 1], f32, tag="syyi", name="syyi")
            nc.vector.tensor_mul(syyi, a_sb, rdet)   # quad coeff for dmy^2
            bi2 = sm.tile([CH, 1], f32, tag="bi2", name="bi2")
            nc.vector.tensor_mul(bi2, sxy, rdet)
            nc.scalar.mul(bi2, bi2, 2.0)             # 2*b/det

            # dm (sign-flipped; cancels in quad)
            dmx = sm.tile([CH, NB], f32, tag="dmx", name="dmx")
            nc.vector.tensor_scalar_sub(dmx, musx_sb, mux[:, 0:1])
            dmy = sm.tile([CH, NB], f32, tag="dmy", name="dmy")
            nc.vector.tensor_scalar_sub(dmy, musy_sb, muy[:, 0:1])

            quad = sm.tile([CH, NB], f32, tag="quad", name="quad")
            tnb = sm.tile([CH, NB], f32, tag="tnb", name="tnb")
            nc.vector.tensor_mul(tnb, dmx, dmx)
            nc.vector.tensor_scalar_mul(quad, tnb, sxxi[:, 0:1])
            nc.vector.tensor_mul(tnb, dmy, dmy)
            nc.vector.tensor_scalar_mul(tnb, tnb, syyi[:, 0:1])
            nc.vector.tensor_add(quad, quad, tnb)
            nc.vector.tensor_mul(tnb, dmx, dmy)
            nc.vector.tensor_scalar_mul(tnb, tnb, bi2[:, 0:1])
            nc.vector.tensor_sub(quad, quad, tnb)

            # r = exp(-0.5*quad) / (2*pi*sqrt(det))
            sqd = sm.tile([CH, 1], f32, tag="sqd", name="sqd")
            nc.scalar.activation(
                out=sqd, in_=det, func=AF.Sqrt, bias=zbias[:, 0:1], scale=1.0
            )
            nc.scalar.mul(sqd, sqd, TWO_PI)
            cc = sm.tile([CH, 1], f32, tag="cc", name="cc")
            nc.vector.reciprocal(out=cc, in_=sqd)
            r_sb = sm.tile([CH, NB], f32, tag="r_sb", name="r_sb")
            nc.scalar.activation(
                out=r_sb, in_=quad, func=AF.Exp, bias=zbias[:, 0:1], scale=-0.5
            )
            nc.vector.tensor_scalar_mul(r_sb, r_sb, cc[:, 0:1])

            # ---- w = r @ G^T  (via rT transpose + matmul) ----
            prt = psum_s.tile([NB, CH], f32, tag="sps", name="prt")
            nc.tensor.transpose(out=prt, in_=r_sb, identity=ident[:CH, :CH])
            rT_sb = sm.tile([NB, CH], f32r, tag="rT", name="rT_sb")
            nc.vector.tensor_copy(out=rT_sb, in_=prt)
            pw = psum_s.tile([CH, L], f32, tag="sps", name="pw")
            nc.tensor.matmul(pw, rT_sb, Gt_sb, start=True, stop=True)
            w_sb = sm.tile([CH, L], f32, tag="w_sb", name="w_sb")
            nc.vector.tensor_copy(out=w_sb, in_=pw)

            # ---- wdiag = broadcast(w) * blockmask; Wsel = wdiag^T ----
            wdiag = sm.tile([CH, CROWS], f32, tag="wdiag", name="wdiag")
            w_bcast = bass.AP(
                tensor=w_sb.tensor,
                offset=w_sb.offset,
                ap=[w_sb.ap[0], [0, CH], list(w_sb.ap[-1])],
            )
            nc.vector.tensor_mul(wdiag, w_bcast, bmask_sb)
            Wsel = sm.tile([128, NKT, CH], f32r, tag="Wsel", name="Wsel", bufs=2)
            for kt in range(NKT):
                rows = 128 if kt < NKT - 1 else TAIL
                pws = psum_s.tile([128, CH], f32, tag="sps", name="pws")
                nc.tensor.transpose(
                    out=pws[:rows, :],
                    in_=wdiag[:, kt * 128:kt * 128 + rows],
                    identity=ident[:CH, :CH],
                )
                nc.vector.tensor_copy(out=Wsel[:rows, kt, :], in_=pws[:rows, :])

            # ---- ctx[s, d] = sum_rows Wsel[row, s] * x[row, d] ----
            ctx_sb = sm.tile([CH, H], f32, tag="ctx_sb", name="ctx_sb", bufs=2)
            for nb2 in range(2):
                pc = psum_c.tile([BPC, 512], f32, tag="ctxps", name="pc")
                for kt in range(NKT):
                    rows = xts[kt][1]
                    nc.tensor.matmul(
                        pc[:CH, :],
                        Wsel[:rows, kt, :],
                        xts[kt][0][:rows, nb2 * 512:(nb2 + 1) * 512],
                        start=(kt == 0),
                        stop=(kt == NKT - 1),
                    )
                nc.scalar.copy(
                    out=ctx_sb[:, nb2 * 512:(nb2 + 1) * 512], in_=pc[:CH, :]
                )

            # ---- accumulate ctx^T into ctxT_sb ----
            for hb in range(8):
                pt = psum_s.tile([128, CH], f32, tag="sps", name="pt")
                nc.tensor.transpose(
                    out=pt,
                    in_=ctx_sb[:, hb * 128:(hb + 1) * 128],
                    identity=ident[:CH, :CH],
                )
                nc.vector.tensor_copy(
                    out=ctxT_sb[:, hb, c * CH:(c + 1) * CH], in_=pt
                )

        # ---- merge: out = ctx @ Wm + bm ----
        for nb4 in range(4):
            po = psum_c.tile([BPC, 512], f32, tag="ctxps", name="po")
            for k in range(8):
                nc.tensor.matmul(
                    po[:, :],
                    ctxT_sb[:, k, :],
                    Wm_sb[:, k, nb4 * 512:(nb4 + 1) * 512],
                    start=(k == 0),
                    stop=False,
                )
            nc.tensor.matmul(
                po[:, :],
                ones_sb,
                bm_sb[:, nb4 * 512:(nb4 + 1) * 512],
                start=False,
                stop=True,
            )
            out_sb = sm.tile([BPC, 512], f32, tag="outsb", name="out_sb", bufs=2)
            nc.scalar.copy(out=out_sb, in_=po)
            nc.sync.dma_start(
                out=out_d[:, nb4 * 512:(nb4 + 1) * 512], in_=out_sb
            )

    nc.finalize()
    _MODULE_CACHE["nc"] = nc
    return nc


def _prep_in_maps(x, x_mask, W1, b1, W2, b2, Wm, bm):
    x = np.ascontiguousarray(np.asarray(x, dtype=_F32))
    x_mask = np.asarray(x_mask)
    W1 = np.ascontiguousarray(np.asarray(W1, dtype=_F32))
    b1 = np.asarray(b1, dtype=_F32)
    W2 = np.asarray(W2, dtype=_F32)
    Wm = np.ascontiguousarray(np.asarray(Wm, dtype=_F32))
    bm = np.asarray(bm, dtype=_F32)

    negmask = np.where(x_mask, _F32(-1e9), _F32(0.0)).astype(_F32)  # [B, L]
    b1r = np.ascontiguousarray(b1.reshape(4, 128).T)                # [128, 4]
    W2r = np.ascontiguousarray(W2.reshape(4, 128, 1)[:, :, 0].T)    # [128, 4]
    bmr = np.ascontiguousarray(bm.reshape(1, O))

    in_maps = []
    for c in range(NCORES):
        sl = slice(c * BPC, (c + 1) * BPC)
        in_maps.append(
            {
                "x": np.ascontiguousarray(x[sl].reshape(ROWS, H)),
                "negmask": np.ascontiguousarray(negmask[sl]),
                "W1": W1,
                "b1r": b1r,
                "W2r": W2r,
                "Wm": Wm,
                "bm": bmr,
            }
        )
    return in_maps


LAST_EXEC_NS = None
LAST_TRACE = None


def kernel(x, x_mask, W1, b1, W2, b2, Wm, bm):
    global LAST_EXEC_NS, LAST_TRACE
    nc = _build_module()
    in_maps = _prep_in_maps(x, x_mask, W1, b1, W2, b2, Wm, bm)

    from concourse.bass_utils import run_bass_kernel_spmd

    res = run_bass_kernel_spmd(
        nc, in_maps, core_ids=list(range(NCORES)), trace=False
    )
    LAST_EXEC_NS = res.exec_time_ns
    LAST_TRACE = res.instructions_and_trace
    out = np.concatenate([r["out"] for r in res.results], axis=0)
    return out.astype(_F32)
